# revision 2
# baseline (speedup 1.0000x reference)
"""DiscreteHazardLoss Trainium2 kernel.

Math
----
reference:  loss_b = -( sum_{j<t} log(1-h_j+eps) + [e=1] log(h_t+eps)
                        + [e=0] log(1-h_t+eps) ),  h = sigmoid(x),  mean over b.
With  log(1-h+eps) ~= -softplus(x)  (eps=1e-7 shift is ~1e-7 relative on the
mean, far below fp32 noise) and  softplus(-x) = softplus(x) - x:

    loss_b = sum_{j<=t_b} softplus(x_bj) - e_b * x_{b,t_b}

Key structural fact: the heavy term only touches elements with j <= t_b
(~16.5 of 32 columns per row on average), and it is a *flat sum* -- row
boundaries are irrelevant.  So the host packs exactly those elements,
contiguously, into one fixed-size fp16 buffer per core (padded with -88,
whose softplus is exactly 0), and the device kernel degenerates to:

    DMA flat buffer -> ACT exp -> ACT ln(e+1) with accum_out -> tiny DMA out

No masks, no DVE work, half the elements, half-again the bytes from fp16.
Exp and Ln share the natural_log_exp_and_others ACT table set (one load).
The event term sum_b e_b*x_{b,t_b} is summed on-device too: the host packs
x_{b,t_b} for e_b=1 rows into a small buffer, the device sums it with an
Identity+accum pass.  Any overflow beyond the fixed buffer capacities
(counts are data-dependent; capacity is mean + ~7 sigma) is finished
exactly on the host in float64, so the kernel is correct for any input.

Sharding: pure data-parallel over the batch axis, 8 cores, 262144 rows each.
fp16 quantization of x is far below the 2e-2 tolerance (errors are random,
~5e-4 relative per element, and average out over 34M summed terms).
"""

import sys

for _p in ("/opt/trn_rl_repo",):
    if _p not in sys.path:
        sys.path.insert(0, _p)

import numpy as np
from contextlib import ExitStack

import concourse.bass as bass
import concourse.bacc as bacc
import concourse.tile as tile
import concourse.mybir as mybir
from concourse.bass_utils import run_bass_kernel_spmd

B, T = 2097152, 32
NCORES = 8
P = 128                      # SBUF partitions
ROWS_PC = B // NCORES        # 262144 rows per core
NT = 8                       # tiles per core
FD = 4256                    # free-dim elems per partition per tile
FDTOT = NT * FD              # 34048 elems per partition
E_FIX = P * FDTOT            # 4,358,144 elems per core (mean 4,325,376)
EV_FD = 1088                 # event-term elems per partition
EV_CAP = P * EV_FD           # 139,264 (mean 131,072, sd 256)
PAD = np.float16(-88.0)      # softplus(-88) == 0 exactly in fp16

_CACHE = {}


def _build_nc(repeat=1):
    nc = bacc.Bacc(
        "TRN2",
        target_bir_lowering=False,
        debug=False,
        enable_asserts=False,
        num_devices=NCORES,
    )
    x_d = nc.dram_tensor("packed", [NT, P, FD], mybir.dt.float16, kind="ExternalInput")
    ev_d = nc.dram_tensor("events_x", [P, EV_FD], mybir.dt.float16, kind="ExternalInput")
    acc_d = nc.dram_tensor("acc", [P, NT + 1], mybir.dt.float32, kind="ExternalOutput")

    with tile.TileContext(nc) as tc, ExitStack() as ctx:
        pool = ctx.enter_context(tc.tile_pool(name="work", bufs=3))
        singles = ctx.enter_context(tc.tile_pool(name="singles", bufs=1))

        acc_tile = singles.tile([P, NT + 1], mybir.dt.float32)

        evt = singles.tile([P, EV_FD], mybir.dt.float16)
        nc.sync.dma_start(out=evt, in_=ev_d.ap())

        for i in range(NT * repeat):
            n = i % NT
            xt = pool.tile([P, FD], mybir.dt.float16, tag="x", bufs=4)
            nc.sync.dma_start(out=xt, in_=x_d.ap()[n])

            e_t = pool.tile([P, FD], mybir.dt.float16, tag="e")
            nc.scalar.activation(
                out=e_t, in_=xt, func=mybir.ActivationFunctionType.Exp
            )

            lnout = pool.tile([P, FD], mybir.dt.float16, tag="ln")
            nc.scalar.activation(
                out=lnout,
                in_=e_t,
                func=mybir.ActivationFunctionType.Ln,
                bias=1.0,
                accum_out=acc_tile[:, n : n + 1],
            )

        # event-term sum, last so the loaded Exp/Ln table set (which contains
        # Identity as filler) keeps serving and no extra table load happens
        evo = singles.tile([P, EV_FD], mybir.dt.float16)
        nc.scalar.activation(
            out=evo,
            in_=evt,
            func=mybir.ActivationFunctionType.Identity,
            accum_out=acc_tile[:, NT : NT + 1],
        )

        nc.sync.dma_start(out=acc_d.ap(), in_=acc_tile)

    # Exp and Ln share one ACT table set; without this the compiler may
    # alternate exp_and_others / natural_log per tile (~2.7us per reload).
    _orig_tables = bacc.get_activation_tables

    def _pinned_tables(arch):
        exp_ln = {
            mybir.ActivationFunctionType.Exp,
            mybir.ActivationFunctionType.Ln,
        }
        return {
            name: (funcs if name == "natural_log_exp_and_others" else funcs - exp_ln)
            for name, funcs in _orig_tables(arch).items()
        }

    bacc.get_activation_tables = _pinned_tables
    try:
        nc.compile()
    finally:
        bacc.get_activation_tables = _orig_tables
    return nc


def _get_nc(repeat=1):
    key = ("nc", repeat)
    if key not in _CACHE:
        _CACHE[key] = _build_nc(repeat)
    return _CACHE[key]


def _pack(logits, time_bins, events):
    """Host-side: extract the j<=t prefix elements and event-bin values per
    core shard into fixed-size fp16 buffers; return per-core input maps plus
    exact float64 corrections for anything beyond the fixed capacities."""
    t = np.clip(np.asarray(time_bins), 0, T - 1).astype(np.int64)
    ev = np.asarray(events).astype(bool)
    xh = np.asarray(logits, dtype=np.float16)
    cols = np.arange(T, dtype=np.int64)

    in_maps = []
    host_sp = 0.0     # softplus tail beyond E_FIX (adds to loss total)
    host_ev = 0.0     # event tail beyond EV_CAP (subtracted from loss total)
    for c in range(NCORES):
        sl = slice(c * ROWS_PC, (c + 1) * ROWS_PC)
        tc_, xc, evc = t[sl], xh[sl], ev[sl]

        flat = xc[cols[None, :] <= tc_[:, None]]
        cnt = min(flat.shape[0], E_FIX)
        if flat.shape[0] > E_FIX:
            tail = flat[E_FIX:].astype(np.float64)
            host_sp += np.log1p(np.exp(tail)).sum()
        buf = np.full(E_FIX, PAD, dtype=np.float16)
        buf[:cnt] = flat[:cnt]

        vals = xc[np.nonzero(evc)[0], tc_[evc]]
        ecnt = min(vals.shape[0], EV_CAP)
        if vals.shape[0] > EV_CAP:
            host_ev += vals[EV_CAP:].astype(np.float64).sum()
        ebuf = np.zeros(EV_CAP, dtype=np.float16)
        ebuf[:ecnt] = vals[:ecnt]

        in_maps.append(
            {
                "packed": buf.reshape(NT, P, FD),
                "events_x": ebuf.reshape(P, EV_FD),
            }
        )
    return in_maps, host_sp, host_ev


def kernel(logits, time_bins, events):
    in_maps, host_sp, host_ev = _pack(logits, time_bins, events)
    nc = _get_nc()
    res = run_bass_kernel_spmd(nc, in_maps, core_ids=list(range(NCORES)))

    total = host_sp - host_ev
    for c in range(NCORES):
        a = res.results[c]["acc"].astype(np.float64)
        total += a[:, :NT].sum() - a[:, NT].sum()
    return np.float32(total / B)


# revision 7
# speedup vs baseline: 1.3001x; 1.3001x over previous
"""DiscreteHazardLoss Trainium2 kernel.

Math
----
reference:  loss_b = -( sum_{j<t} log(1-h_j+eps) + [e=1] log(h_t+eps)
                        + [e=0] log(1-h_t+eps) ),  h = sigmoid(x),  mean over b.
With  log(1-h+eps) ~= -softplus(x)  (eps=1e-7 shift is ~1e-7 relative on the
mean, far below fp32 noise) and  softplus(-x) = softplus(x) - x:

    loss_b = sum_{j<=t_b} softplus(x_bj) - e_b * x_{b,t_b}

Key structural fact: the heavy term only touches elements with j <= t_b
(~16.5 of 32 columns per row on average), and it is a *flat sum* -- row
boundaries are irrelevant.  So the host packs exactly those elements,
contiguously, into one fixed-size fp16 buffer per core (padded with -88,
whose softplus is exactly 0), and the device kernel degenerates to:

    DMA flat buffer -> ACT exp -> ACT ln(e+1) with accum_out -> tiny DMA out

No masks, no DVE work, half the elements, half-again the bytes from fp16.
Exp and Ln share the natural_log_exp_and_others ACT table set (one load).
The event term sum_b e_b*x_{b,t_b} is summed on-device too: the host packs
x_{b,t_b} for e_b=1 rows into a small buffer, the device sums it with an
Identity+accum pass.  Any overflow beyond the fixed buffer capacities
(counts are data-dependent; capacity is mean + ~7 sigma) is finished
exactly on the host in float64, so the kernel is correct for any input.

Sharding: pure data-parallel over the batch axis, 8 cores, 262144 rows each.
fp16 quantization of x is far below the 2e-2 tolerance (errors are random,
~5e-4 relative per element, and average out over 34M summed terms).
"""

import os
import sys

for _p in ("/opt/trn_rl_repo",):
    if _p not in sys.path:
        sys.path.insert(0, _p)

import numpy as np
from contextlib import ExitStack

import concourse.bass as bass
import concourse.bacc as bacc
import concourse.tile as tile
import concourse.mybir as mybir
from concourse.bass_utils import run_bass_kernel_spmd

B, T = 2097152, 32
NCORES = 8
P = 128                      # SBUF partitions
ROWS_PC = B // NCORES        # 262144 rows per core
NT = int(os.environ.get("KNT", "8"))   # tiles per core
FDTOT = 34048                # elems per partition (NT*FD)
FD = FDTOT // NT             # free-dim elems per partition per tile
E_FIX = P * FDTOT            # 4,358,144 elems per core (mean 4,325,376)
EV_FD = 1088                 # event-term elems per partition
EV_CAP = P * EV_FD           # 139,264 (mean 131,072, sd 256)
PAD = -88.0                  # softplus(-88) == 0; exact in fp16/bf16/fp8e4

# device dtypes (sweepable): packed input / exp output / ln output
IN_DT = getattr(mybir.dt, os.environ.get("KDT_IN", "float16"))
MID_DT = getattr(mybir.dt, os.environ.get("KDT_MID", "float16"))
OUT_DT = getattr(mybir.dt, os.environ.get("KDT_OUT", "float16"))
IN_NP = mybir.dt.np(IN_DT)

_CACHE = {}


def _build_nc(repeat=1):
    nc = bacc.Bacc(
        "TRN2",
        target_bir_lowering=False,
        debug=False,
        enable_asserts=False,
        num_devices=NCORES,
    )
    x_d = nc.dram_tensor("packed", [NT, P, FD], IN_DT, kind="ExternalInput")
    ev_d = nc.dram_tensor("events_x", [P, EV_FD], mybir.dt.float16, kind="ExternalInput")
    acc_d = nc.dram_tensor("acc", [P, NT + 1], mybir.dt.float32, kind="ExternalOutput")

    with tile.TileContext(nc) as tc, ExitStack() as ctx:
        pool = ctx.enter_context(tc.tile_pool(name="work", bufs=3))
        singles = ctx.enter_context(tc.tile_pool(name="singles", bufs=1))

        acc_tile = singles.tile([P, NT + 1], mybir.dt.float32)

        evt = singles.tile([P, EV_FD], mybir.dt.float16)
        nc.sync.dma_start(out=evt, in_=ev_d.ap())

        for i in range(NT * repeat):
            n = i % NT
            xt = pool.tile([P, FD], IN_DT, tag="x", bufs=4)
            nc.sync.dma_start(out=xt, in_=x_d.ap()[n])

            e_t = pool.tile([P, FD], MID_DT, tag="e")
            nc.scalar.activation(
                out=e_t, in_=xt, func=mybir.ActivationFunctionType.Exp
            )

            lnout = pool.tile([P, FD], OUT_DT, tag="ln")
            nc.scalar.activation(
                out=lnout,
                in_=e_t,
                func=mybir.ActivationFunctionType.Ln,
                bias=1.0,
                accum_out=acc_tile[:, n : n + 1],
            )

        # event-term sum, last so the loaded Exp/Ln table set (which contains
        # Identity as filler) keeps serving and no extra table load happens
        evo = singles.tile([P, EV_FD], mybir.dt.float16)
        nc.scalar.activation(
            out=evo,
            in_=evt,
            func=mybir.ActivationFunctionType.Identity,
            accum_out=acc_tile[:, NT : NT + 1],
        )

        nc.sync.dma_start(out=acc_d.ap(), in_=acc_tile)

    # Exp and Ln share one ACT table set; without this the compiler may
    # alternate exp_and_others / natural_log per tile (~2.7us per reload).
    _orig_tables = bacc.get_activation_tables

    def _pinned_tables(arch):
        exp_ln = {
            mybir.ActivationFunctionType.Exp,
            mybir.ActivationFunctionType.Ln,
        }
        return {
            name: (funcs if name == "natural_log_exp_and_others" else funcs - exp_ln)
            for name, funcs in _orig_tables(arch).items()
        }

    bacc.get_activation_tables = _pinned_tables
    try:
        nc.compile()
    finally:
        bacc.get_activation_tables = _orig_tables
    return nc


def _get_nc(repeat=1):
    key = ("nc", repeat)
    if key not in _CACHE:
        _CACHE[key] = _build_nc(repeat)
    return _CACHE[key]


def _pack(logits, time_bins, events):
    """Host-side: extract the j<=t prefix elements and event-bin values per
    core shard into fixed-size fp16 buffers; return per-core input maps plus
    exact float64 corrections for anything beyond the fixed capacities."""
    t = np.clip(np.asarray(time_bins), 0, T - 1).astype(np.int64)
    ev = np.asarray(events).astype(bool)
    xh = np.asarray(logits, dtype=IN_NP)
    xh16 = xh if IN_NP == np.float16 else np.asarray(logits, dtype=np.float16)
    cols = np.arange(T, dtype=np.int64)

    in_maps = []
    host_sp = 0.0     # softplus tail beyond E_FIX (adds to loss total)
    host_ev = 0.0     # event tail beyond EV_CAP (subtracted from loss total)
    for c in range(NCORES):
        sl = slice(c * ROWS_PC, (c + 1) * ROWS_PC)
        tc_, xc, evc = t[sl], xh[sl], ev[sl]

        flat = xc[cols[None, :] <= tc_[:, None]]
        cnt = min(flat.shape[0], E_FIX)
        if flat.shape[0] > E_FIX:
            tail = flat[E_FIX:].astype(np.float64)
            host_sp += np.log1p(np.exp(tail)).sum()
        buf = np.full(E_FIX, PAD, dtype=IN_NP)
        buf[:cnt] = flat[:cnt]

        vals = xh16[sl][np.nonzero(evc)[0], tc_[evc]]
        ecnt = min(vals.shape[0], EV_CAP)
        if vals.shape[0] > EV_CAP:
            host_ev += vals[EV_CAP:].astype(np.float64).sum()
        ebuf = np.zeros(EV_CAP, dtype=np.float16)
        ebuf[:ecnt] = vals[:ecnt]

        in_maps.append(
            {
                "packed": buf.reshape(NT, P, FD),
                "events_x": ebuf.reshape(P, EV_FD),
            }
        )
    return in_maps, host_sp, host_ev


def kernel(logits, time_bins, events):
    in_maps, host_sp, host_ev = _pack(logits, time_bins, events)
    nc = _get_nc()
    res = run_bass_kernel_spmd(nc, in_maps, core_ids=list(range(NCORES)))

    total = host_sp - host_ev
    for c in range(NCORES):
        a = res.results[c]["acc"].astype(np.float64)
        total += a[:, :NT].sum() - a[:, NT].sum()
    return np.float32(total / B)


# revision 9
# speedup vs baseline: 1.8233x; 1.4024x over previous
"""DiscreteHazardLoss Trainium2 kernel.

Math
----
reference:  loss_b = -( sum_{j<t} log(1-h_j+eps) + [e=1] log(h_t+eps)
                        + [e=0] log(1-h_t+eps) ),  h = sigmoid(x),  mean over b.
With  log(1-h+eps) ~= -softplus(x)  (eps=1e-7 shift is ~1e-7 relative on the
mean, far below fp32 noise) and  softplus(-x) = softplus(x) - x:

    loss_b = sum_{j<=t_b} softplus(x_bj) - e_b * x_{b,t_b}

Key structural fact: the heavy term only touches elements with j <= t_b
(~16.5 of 32 columns per row on average), and it is a *flat sum* -- row
boundaries are irrelevant.  So the host packs exactly those elements,
contiguously, into one fixed-size fp16 buffer per core (padded with -88,
whose softplus is exactly 0), and the device kernel degenerates to:

    DMA flat buffer -> ACT exp -> ACT ln(e+1) with accum_out -> tiny DMA out

No masks, no DVE work, half the elements, half-again the bytes from fp16.
Exp and Ln share the natural_log_exp_and_others ACT table set (one load).
The event term sum_b e_b*x_{b,t_b} is summed on-device too: the host packs
x_{b,t_b} for e_b=1 rows into a small buffer, the device sums it with an
Identity+accum pass.  Any overflow beyond the fixed buffer capacities
(counts are data-dependent; capacity is mean + ~7 sigma) is finished
exactly on the host in float64, so the kernel is correct for any input.

Sharding: pure data-parallel over the batch axis, 8 cores, 262144 rows each.
fp16 quantization of x is far below the 2e-2 tolerance (errors are random,
~5e-4 relative per element, and average out over 34M summed terms).
"""

import os
import sys

for _p in ("/opt/trn_rl_repo",):
    if _p not in sys.path:
        sys.path.insert(0, _p)

import numpy as np
from contextlib import ExitStack

import concourse.bass as bass
import concourse.bacc as bacc
import concourse.tile as tile
import concourse.mybir as mybir
from concourse.bass_utils import run_bass_kernel_spmd

B, T = 2097152, 32
NCORES = 8
P = 128                      # SBUF partitions
ROWS_PC = B // NCORES        # 262144 rows per core
NT = int(os.environ.get("KNT", "8"))   # tiles per core
FDTOT = 34048                # elems per partition (NT*FD)
FD = FDTOT // NT             # free-dim elems per partition per tile
E_FIX = P * FDTOT            # 4,358,144 elems per core (mean 4,325,376)
EV_FD = 1088                 # event-term elems per partition
EV_CAP = P * EV_FD           # 139,264 (mean 131,072, sd 256)
PAD = -88.0                  # softplus(-88) == 0; exact in fp16/bf16/fp8e4

# device dtypes (sweepable): packed input / exp output / ln output
IN_DT = getattr(mybir.dt, os.environ.get("KDT_IN", "float16"))
MID_DT = getattr(mybir.dt, os.environ.get("KDT_MID", "float16"))
OUT_DT = getattr(mybir.dt, os.environ.get("KDT_OUT", "float16"))
IN_NP = mybir.dt.np(IN_DT)
KMODE = os.environ.get("KMODE", "full")  # full | act (no per-tile DMA) | dma (no ACT)

_CACHE = {}


def _build_nc(repeat=1):
    nc = bacc.Bacc(
        "TRN2",
        target_bir_lowering=False,
        debug=False,
        enable_asserts=False,
        num_devices=NCORES,
    )
    x_d = nc.dram_tensor("packed", [NT, P, FD], IN_DT, kind="ExternalInput")
    ev_d = nc.dram_tensor("events_x", [P, EV_FD], mybir.dt.float16, kind="ExternalInput")
    acc_d = nc.dram_tensor("acc", [P, NT + 1], mybir.dt.float32, kind="ExternalOutput")

    with tile.TileContext(nc) as tc, ExitStack() as ctx:
        pool = ctx.enter_context(tc.tile_pool(name="work", bufs=3))
        singles = ctx.enter_context(tc.tile_pool(name="singles", bufs=1))

        acc_tile = singles.tile([P, NT + 1], mybir.dt.float32)

        evt = singles.tile([P, EV_FD], mybir.dt.float16)
        nc.sync.dma_start(out=evt, in_=ev_d.ap())

        x_fixed = None
        if KMODE == "act":
            x_fixed = singles.tile([P, FD], IN_DT)
            nc.sync.dma_start(out=x_fixed, in_=x_d.ap()[0])

        for i in range(NT * repeat):
            n = i % NT
            if KMODE == "act":
                xt = x_fixed
            else:
                xt = pool.tile([P, FD], IN_DT, tag="x", bufs=4)
                nc.sync.dma_start(out=xt, in_=x_d.ap()[n])
            if KMODE == "dma":
                continue

            e_t = pool.tile([P, FD], MID_DT, tag="e")
            nc.scalar.activation(
                out=e_t, in_=xt, func=mybir.ActivationFunctionType.Exp
            )

            lnout = pool.tile([P, FD], OUT_DT, tag="ln")
            nc.scalar.activation(
                out=lnout,
                in_=e_t,
                func=mybir.ActivationFunctionType.Ln,
                bias=1.0,
                accum_out=acc_tile[:, n : n + 1],
            )

        # event-term sum, last so the loaded Exp/Ln table set (which contains
        # Identity as filler) keeps serving and no extra table load happens
        evo = singles.tile([P, EV_FD], mybir.dt.float16)
        nc.scalar.activation(
            out=evo,
            in_=evt,
            func=mybir.ActivationFunctionType.Identity,
            accum_out=acc_tile[:, NT : NT + 1],
        )

        nc.sync.dma_start(out=acc_d.ap(), in_=acc_tile)

    # Exp and Ln share one ACT table set; without this the compiler may
    # alternate exp_and_others / natural_log per tile (~2.7us per reload).
    _orig_tables = bacc.get_activation_tables

    def _pinned_tables(arch):
        exp_ln = {
            mybir.ActivationFunctionType.Exp,
            mybir.ActivationFunctionType.Ln,
        }
        return {
            name: (funcs if name == "natural_log_exp_and_others" else funcs - exp_ln)
            for name, funcs in _orig_tables(arch).items()
        }

    bacc.get_activation_tables = _pinned_tables
    try:
        nc.compile()
    finally:
        bacc.get_activation_tables = _orig_tables
    return nc


def _get_nc(repeat=1):
    key = ("nc", repeat)
    if key not in _CACHE:
        _CACHE[key] = _build_nc(repeat)
    return _CACHE[key]


def _pack(logits, time_bins, events):
    """Host-side: extract the j<=t prefix elements and event-bin values per
    core shard into fixed-size fp16 buffers; return per-core input maps plus
    exact float64 corrections for anything beyond the fixed capacities."""
    t = np.clip(np.asarray(time_bins), 0, T - 1).astype(np.int64)
    ev = np.asarray(events).astype(bool)
    xh = np.asarray(logits, dtype=IN_NP)
    xh16 = xh if IN_NP == np.float16 else np.asarray(logits, dtype=np.float16)
    cols = np.arange(T, dtype=np.int64)

    in_maps = []
    host_sp = 0.0     # softplus tail beyond E_FIX (adds to loss total)
    host_ev = 0.0     # event tail beyond EV_CAP (subtracted from loss total)
    for c in range(NCORES):
        sl = slice(c * ROWS_PC, (c + 1) * ROWS_PC)
        tc_, xc, evc = t[sl], xh[sl], ev[sl]

        flat = xc[cols[None, :] <= tc_[:, None]]
        cnt = min(flat.shape[0], E_FIX)
        if flat.shape[0] > E_FIX:
            tail = flat[E_FIX:].astype(np.float64)
            host_sp += np.log1p(np.exp(tail)).sum()
        buf = np.full(E_FIX, PAD, dtype=IN_NP)
        buf[:cnt] = flat[:cnt]

        vals = xh16[sl][np.nonzero(evc)[0], tc_[evc]]
        ecnt = min(vals.shape[0], EV_CAP)
        if vals.shape[0] > EV_CAP:
            host_ev += vals[EV_CAP:].astype(np.float64).sum()
        ebuf = np.zeros(EV_CAP, dtype=np.float16)
        ebuf[:ecnt] = vals[:ecnt]

        in_maps.append(
            {
                "packed": buf.reshape(NT, P, FD),
                "events_x": ebuf.reshape(P, EV_FD),
            }
        )
    return in_maps, host_sp, host_ev


def kernel(logits, time_bins, events):
    in_maps, host_sp, host_ev = _pack(logits, time_bins, events)
    nc = _get_nc()
    res = run_bass_kernel_spmd(nc, in_maps, core_ids=list(range(NCORES)))

    total = host_sp - host_ev
    for c in range(NCORES):
        a = res.results[c]["acc"].astype(np.float64)
        total += a[:, :NT].sum() - a[:, NT].sum()
    return np.float32(total / B)


# revision 11
# speedup vs baseline: 3.6216x; 1.9863x over previous
"""DiscreteHazardLoss Trainium2 kernel.

Math
----
reference:  loss_b = -( sum_{j<t} log(1-h_j+eps) + [e=1] log(h_t+eps)
                        + [e=0] log(1-h_t+eps) ),  h = sigmoid(x),  mean over b.
With  log(1-h+eps) ~= -softplus(x)  (eps=1e-7 shift is ~1e-7 relative on the
mean, far below fp32 noise) and  softplus(-x) = softplus(x) - x:

    loss_b = sum_{j<=t_b} softplus(x_bj) - e_b * x_{b,t_b}

Key structural fact: the heavy term only touches elements with j <= t_b
(~16.5 of 32 columns per row on average), and it is a *flat sum* -- row
boundaries are irrelevant.  So the host packs exactly those elements,
contiguously, into one fixed-size fp16 buffer per core (padded with -88,
whose softplus is exactly 0), and the device kernel degenerates to:

    DMA flat buffer -> ACT exp -> ACT ln(e+1) with accum_out -> tiny DMA out

No masks, no DVE work, half the elements, half-again the bytes from fp16.
Exp and Ln share the natural_log_exp_and_others ACT table set (one load).
The event term sum_b e_b*x_{b,t_b} is summed on-device too: the host packs
x_{b,t_b} for e_b=1 rows into a small buffer, the device sums it with an
Identity+accum pass.  Any overflow beyond the fixed buffer capacities
(counts are data-dependent; capacity is mean + ~7 sigma) is finished
exactly on the host in float64, so the kernel is correct for any input.

Sharding: pure data-parallel over the batch axis, 8 cores, 262144 rows each.
fp16 quantization of x is far below the 2e-2 tolerance (errors are random,
~5e-4 relative per element, and average out over 34M summed terms).
"""

import os
import sys

for _p in ("/opt/trn_rl_repo",):
    if _p not in sys.path:
        sys.path.insert(0, _p)

import numpy as np
from contextlib import ExitStack

import concourse.bass as bass
import concourse.bacc as bacc
import concourse.tile as tile
import concourse.mybir as mybir
from concourse.bass_utils import run_bass_kernel_spmd

B, T = 2097152, 32
NCORES = 8
P = 128                      # SBUF partitions
ROWS_PC = B // NCORES        # 262144 rows per core
NT = int(os.environ.get("KNT", "8"))   # tiles per core
FDTOT = 34048                # elems per partition (NT*FD)
FD = FDTOT // NT             # free-dim elems per partition per tile
E_FIX = P * FDTOT            # 4,358,144 elems per core (mean 4,325,376)
EV_FD = 1088                 # event-term elems per partition
EV_CAP = P * EV_FD           # 139,264 (mean 131,072, sd 256)
PAD = -88.0                  # softplus(-88) == 0; exact in fp16/bf16/fp8e4

# device dtypes (sweepable): packed input / exp output / ln output
IN_DT = getattr(mybir.dt, os.environ.get("KDT_IN", "float16"))
MID_DT = getattr(mybir.dt, os.environ.get("KDT_MID", "float16"))
OUT_DT = getattr(mybir.dt, os.environ.get("KDT_OUT", "float16"))
IN_NP = mybir.dt.np(IN_DT)
KMODE = os.environ.get("KMODE", "full")  # full | act (no per-tile DMA) | dma (no ACT)

_CACHE = {}


def _build_nc(repeat=1):
    nc = bacc.Bacc(
        "TRN2",
        target_bir_lowering=False,
        debug=False,
        enable_asserts=False,
        num_devices=NCORES,
    )
    x_d = nc.dram_tensor("packed", [NT, P, FD], IN_DT, kind="ExternalInput")
    ev_d = nc.dram_tensor("events_x", [P, EV_FD], mybir.dt.float16, kind="ExternalInput")
    acc_d = nc.dram_tensor("acc", [P, NT + 1], mybir.dt.float32, kind="ExternalOutput")

    with tile.TileContext(nc) as tc, ExitStack() as ctx:
        pool = ctx.enter_context(tc.tile_pool(name="work", bufs=3))
        singles = ctx.enter_context(tc.tile_pool(name="singles", bufs=1))

        acc_tile = singles.tile([P, NT + 1], mybir.dt.float32)

        evt = singles.tile([P, EV_FD], mybir.dt.float16)
        nc.sync.dma_start(out=evt, in_=ev_d.ap())

        x_fixed = None
        if KMODE == "act":
            x_fixed = singles.tile([P, FD], IN_DT)
            nc.sync.dma_start(out=x_fixed, in_=x_d.ap()[0])

        for i in range(NT * repeat):
            n = i % NT
            if KMODE == "act":
                xt = x_fixed
            else:
                xt = pool.tile([P, FD], IN_DT, tag="x", bufs=4)
                nc.sync.dma_start(out=xt, in_=x_d.ap()[n])
            if KMODE == "dma":
                continue

            e_t = pool.tile([P, FD], MID_DT, tag="e")
            nc.scalar.activation(
                out=e_t, in_=xt, func=mybir.ActivationFunctionType.Exp
            )

            lnout = pool.tile([P, FD], OUT_DT, tag="ln")
            nc.scalar.activation(
                out=lnout,
                in_=e_t,
                func=mybir.ActivationFunctionType.Ln,
                bias=1.0,
                accum_out=acc_tile[:, n : n + 1],
            )

        # event-term sum, last so the loaded Exp/Ln table set (which contains
        # Identity as filler) keeps serving and no extra table load happens
        evo = singles.tile([P, EV_FD], mybir.dt.float16)
        nc.scalar.activation(
            out=evo,
            in_=evt,
            func=mybir.ActivationFunctionType.Identity,
            accum_out=acc_tile[:, NT : NT + 1],
        )

        if KMODE == "dma":
            # acc_tile is never written by the loop in this mode; give it a
            # real producer so the out-DMA's dependency can be satisfied
            nc.scalar.activation(
                out=acc_tile,
                in_=evt[:, : NT + 1],
                func=mybir.ActivationFunctionType.Copy,
            )
        nc.sync.dma_start(out=acc_d.ap(), in_=acc_tile)

    # Exp and Ln share one ACT table set; without this the compiler may
    # alternate exp_and_others / natural_log per tile (~2.7us per reload).
    _orig_tables = bacc.get_activation_tables

    def _pinned_tables(arch):
        exp_ln = {
            mybir.ActivationFunctionType.Exp,
            mybir.ActivationFunctionType.Ln,
        }
        return {
            name: (funcs if name == "natural_log_exp_and_others" else funcs - exp_ln)
            for name, funcs in _orig_tables(arch).items()
        }

    bacc.get_activation_tables = _pinned_tables
    try:
        nc.compile()
    finally:
        bacc.get_activation_tables = _orig_tables
    return nc


def _get_nc(repeat=1):
    key = ("nc", repeat)
    if key not in _CACHE:
        _CACHE[key] = _build_nc(repeat)
    return _CACHE[key]


def _pack(logits, time_bins, events):
    """Host-side: extract the j<=t prefix elements and event-bin values per
    core shard into fixed-size fp16 buffers; return per-core input maps plus
    exact float64 corrections for anything beyond the fixed capacities."""
    t = np.clip(np.asarray(time_bins), 0, T - 1).astype(np.int64)
    ev = np.asarray(events).astype(bool)
    xh = np.asarray(logits, dtype=IN_NP)
    xh16 = xh if IN_NP == np.float16 else np.asarray(logits, dtype=np.float16)
    cols = np.arange(T, dtype=np.int64)

    in_maps = []
    host_sp = 0.0     # softplus tail beyond E_FIX (adds to loss total)
    host_ev = 0.0     # event tail beyond EV_CAP (subtracted from loss total)
    for c in range(NCORES):
        sl = slice(c * ROWS_PC, (c + 1) * ROWS_PC)
        tc_, xc, evc = t[sl], xh[sl], ev[sl]

        flat = xc[cols[None, :] <= tc_[:, None]]
        cnt = min(flat.shape[0], E_FIX)
        if flat.shape[0] > E_FIX:
            tail = flat[E_FIX:].astype(np.float64)
            host_sp += np.log1p(np.exp(tail)).sum()
        buf = np.full(E_FIX, PAD, dtype=IN_NP)
        buf[:cnt] = flat[:cnt]

        vals = xh16[sl][np.nonzero(evc)[0], tc_[evc]]
        ecnt = min(vals.shape[0], EV_CAP)
        if vals.shape[0] > EV_CAP:
            host_ev += vals[EV_CAP:].astype(np.float64).sum()
        ebuf = np.zeros(EV_CAP, dtype=np.float16)
        ebuf[:ecnt] = vals[:ecnt]

        in_maps.append(
            {
                "packed": buf.reshape(NT, P, FD),
                "events_x": ebuf.reshape(P, EV_FD),
            }
        )
    return in_maps, host_sp, host_ev


def kernel(logits, time_bins, events):
    in_maps, host_sp, host_ev = _pack(logits, time_bins, events)
    nc = _get_nc()
    res = run_bass_kernel_spmd(nc, in_maps, core_ids=list(range(NCORES)))

    total = host_sp - host_ev
    for c in range(NCORES):
        a = res.results[c]["acc"].astype(np.float64)
        total += a[:, :NT].sum() - a[:, NT].sum()
    return np.float32(total / B)
